# revision 1
# baseline (speedup 1.0000x reference)
"""Trainium2 Bass kernel for nn_CompressNetwork (grouped per-slot MLP).

Reference computation (per slot m of M=32):
    h   = relu(x[:, m, :] @ W1[m] + b1[m])      # [B, D_H]
    out = h @ W2[m] + b2[m]                     # [B, D_OUT]

Sharding: slots are split across 8 NeuronCores (4 slots/core), expert-parallel.
No cross-core communication. Compute is done in bf16 with fp32 PSUM
accumulation (rel err ~1e-3, well inside tolerance, 4x the fp32 PE rate).

Device-side layout trick: layer 1 computes h^T (h transposed) so that
 - layer 1 stationary operand is W1 in its *natural* [D_IN, D_H] layout,
 - layer 1 moving operand is x^T (pre-transposed on host, free),
 - b1 becomes a per-partition bias => fused into the ScalarE relu,
 - layer 2 stationary operand is a [128,128] slice of h^T (already in SBUF),
 - layer 2 moving operand is W2 in its *natural* [D_H, D_OUT] layout,
so no on-device transposes are needed anywhere.
"""

import numpy as np
import ml_dtypes

import concourse.tile as tile
from concourse import bacc, mybir
from concourse.bass import ds, ts
from concourse.bass_utils import run_bass_kernel_spmd

BF16 = ml_dtypes.bfloat16

B, M, D_IN, D_H, D_OUT = 512, 32, 1024, 4096, 1024
N_CORES = 8
SLOTS = M // N_CORES          # 4 slots per core
P = 128                       # SBUF partitions / PE array size
KI = D_IN // P                # 8 contraction chunks in layer 1
KH = D_H // P                 # 32 contraction chunks in layer 2
NB = B // P                   # 4 output row-blocks in layer 2
F = 512                       # moving free dim == one PSUM bank of fp32
ND = D_OUT // F               # 2 output column halves in layer 2

_NC_CACHE = {}


def build_nc():
    nc = bacc.Bacc("TRN2", target_bir_lowering=False, debug=False,
                   num_devices=N_CORES)
    bf = mybir.dt.bfloat16
    f32 = mybir.dt.float32
    xT = nc.dram_tensor("xT", [SLOTS, P, KI * B], bf, kind="ExternalInput").ap()
    w1 = nc.dram_tensor("w1", [SLOTS, KH, P, KI * P], bf, kind="ExternalInput").ap()
    b1 = nc.dram_tensor("b1", [SLOTS, P, KH], f32, kind="ExternalInput").ap()
    w2 = nc.dram_tensor("w2", [SLOTS, KH, P, D_OUT], bf, kind="ExternalInput").ap()
    b2 = nc.dram_tensor("b2", [SLOTS, P, D_OUT], f32, kind="ExternalInput").ap()
    out = nc.dram_tensor("out", [SLOTS, B, D_OUT], f32, kind="ExternalOutput").ap()

    with tile.TileContext(nc) as tc:
        with (
            tc.tile_pool(name="xt_pool", bufs=2) as xt_pool,
            tc.tile_pool(name="w1_pool", bufs=4) as w1_pool,
            tc.tile_pool(name="w2_pool", bufs=1) as w2_pool,
            tc.tile_pool(name="h_pool", bufs=2) as h_pool,
            tc.tile_pool(name="bias_pool", bufs=2) as bias_pool,
            tc.tile_pool(name="out_pool", bufs=4) as out_pool,
            tc.tile_pool(name="ps1_pool", bufs=3, space="PSUM") as ps1_pool,
            tc.tile_pool(name="ps2_pool", bufs=4, space="PSUM") as ps2_pool,
        ):
            for s in range(SLOTS):
                xt = xt_pool.tile([P, KI * B], bf, name=f"xt_{s}", tag="xt")
                for k in range(KI):
                    nc.sync.dma_start(out=xt[:, ts(k, B)], in_=xT[s, :, ts(k, B)])
                b1t = bias_pool.tile([P, KH], f32, name=f"b1_{s}", tag="b1")
                nc.sync.dma_start(out=b1t[:], in_=b1[s])
                b2t = bias_pool.tile([P, D_OUT], f32, name=f"b2_{s}", tag="b2")
                nc.sync.dma_start(out=b2t[:], in_=b2[s])
                w2t = w2_pool.tile([P, KH * D_OUT], bf, name=f"w2_{s}", tag="w2")
                for j in range(KH):
                    nc.sync.dma_start(out=w2t[:, ts(j, D_OUT)], in_=w2[s, j])

                # Layer 1: h^T[dh, b] = relu(W1^T @ x^T + b1), bf16 out
                h = h_pool.tile([P, KH * B], bf, name=f"h_{s}", tag="h")
                for j in range(KH):
                    w1t = w1_pool.tile([P, KI * P], bf, name=f"w1_{s}_{j}", tag="w1")
                    nc.sync.dma_start(out=w1t[:], in_=w1[s, j])
                    ps1 = ps1_pool.tile([P, B], f32, name=f"ps1_{s}_{j}", tag="ps1")
                    for k in range(KI):
                        nc.tensor.matmul(
                            ps1[:],
                            lhsT=w1t[:, ts(k, P)],
                            rhs=xt[:, ts(k, B)],
                            start=(k == 0),
                            stop=(k == KI - 1),
                        )
                    nc.scalar.activation(
                        h[:, ts(j, B)],
                        ps1[:],
                        mybir.ActivationFunctionType.Relu,
                        bias=b1t[:, ds(j, 1)],
                        scale=1.0,
                    )

                # Layer 2: out[b, o] = h^T-slice^T @ W2 + b2
                for bb in range(NB):
                    pss = [
                        ps2_pool.tile([P, F], f32, name=f"ps2_{s}_{bb}_{d}", tag="ps2")
                        for d in range(ND)
                    ]
                    for j in range(KH):
                        for d in range(ND):
                            nc.tensor.matmul(
                                pss[d][:],
                                lhsT=h[:, ds(j * B + bb * P, P)],
                                rhs=w2t[:, ds(j * D_OUT + d * F, F)],
                                start=(j == 0),
                                stop=(j == KH - 1),
                            )
                    osb = out_pool.tile([P, D_OUT], f32, name=f"osb_{s}_{bb}", tag="osb")
                    for d in range(ND):
                        nc.vector.tensor_tensor(
                            out=osb[:, ts(d, F)],
                            in0=pss[d][:],
                            in1=b2t[:, ts(d, F)],
                            op=mybir.AluOpType.add,
                        )
                    nc.sync.dma_start(out=out[s, ts(bb, P)], in_=osb[:])
    nc.compile()
    return nc


def prep_in_maps(memory_hidden_states, W1, b1, W2, b2):
    """Host-side packing of full inputs into per-core, SBUF-layout arrays."""
    x = np.asarray(memory_hidden_states, dtype=np.float32)
    W1 = np.asarray(W1, dtype=np.float32)
    b1 = np.asarray(b1, dtype=np.float32)
    W2 = np.asarray(W2, dtype=np.float32)
    b2 = np.asarray(b2, dtype=np.float32)

    # x: [B, M, D_IN] -> x^T per slot: [M, P, KI*B]; [m, p, k*B+b] = x[b, m, k*P+p]
    xT = np.ascontiguousarray(
        x.astype(BF16).transpose(1, 2, 0)       # [M, D_IN, B]
         .reshape(M, KI, P, B)
         .transpose(0, 2, 1, 3)                 # [M, P, KI, B]
         .reshape(M, P, KI * B)
    )
    # W1: [M, D_IN, D_H] -> [M, KH, P(=din%128), KI*128]
    w1 = np.ascontiguousarray(
        W1.astype(BF16).reshape(M, KI, P, KH, P)
          .transpose(0, 3, 2, 1, 4)             # [M, KH(j), P(p), KI(k), 128(c)]
          .reshape(M, KH, P, KI * P)
    )
    # b1: [M, D_H] -> [M, P, KH] (per-partition bias per dh-block)
    b1p = np.ascontiguousarray(b1.reshape(M, KH, P).transpose(0, 2, 1))
    # W2: [M, D_H, D_OUT] -> [M, KH, P, D_OUT] (pure reshape)
    w2 = np.ascontiguousarray(W2.astype(BF16).reshape(M, KH, P, D_OUT))
    # b2: [M, D_OUT] -> [M, P, D_OUT] broadcast across partitions
    b2p = np.ascontiguousarray(np.broadcast_to(b2[:, None, :], (M, P, D_OUT)))

    in_maps = []
    for c in range(N_CORES):
        sl = slice(c * SLOTS, (c + 1) * SLOTS)
        in_maps.append({
            "xT": xT[sl],
            "w1": w1[sl],
            "b1": b1p[sl],
            "w2": w2[sl],
            "b2": b2p[sl],
        })
    return in_maps


def assemble_out(results):
    """results: list of 8 dicts with 'out' [SLOTS, B, D_OUT] -> [B, M, D_OUT]."""
    full = np.concatenate([results[c]["out"] for c in range(N_CORES)], axis=0)
    return np.ascontiguousarray(full.transpose(1, 0, 2))


def kernel(memory_hidden_states, W1, b1, W2, b2):
    if "nc" not in _NC_CACHE:
        _NC_CACHE["nc"] = build_nc()
    nc = _NC_CACHE["nc"]
    in_maps = prep_in_maps(memory_hidden_states, W1, b1, W2, b2)
    res = run_bass_kernel_spmd(nc, in_maps, list(range(N_CORES)))
    return assemble_out(res.results)


# revision 4
# speedup vs baseline: 49.9090x; 49.9090x over previous
"""Trainium2 Bass kernel for nn_CompressNetwork (grouped per-slot MLP).

Reference computation (per slot m of M=32):
    h   = relu(x[:, m, :] @ W1[m] + b1[m])      # [B, D_H]
    out = h @ W2[m] + b2[m]                     # [B, D_OUT]

Sharding: slots are split across 8 NeuronCores (4 slots/core), expert-parallel.
No cross-core communication. Compute is done in bf16 with fp32 PSUM
accumulation (rel err ~1e-3, well inside tolerance, 4x the fp32 PE rate).

Device-side layout trick: layer 1 computes h^T (h transposed) so that
 - layer 1 stationary operand is W1 in its *natural* [D_IN, D_H] layout,
 - layer 1 moving operand is x^T (pre-transposed on host, free),
 - b1 becomes a per-partition bias => fused into the ScalarE relu,
 - layer 2 stationary operand is a [128,128] slice of h^T (already in SBUF),
 - layer 2 moving operand is W2 in its *natural* [D_H, D_OUT] layout,
so no on-device transposes are needed anywhere.
"""

import numpy as np
import ml_dtypes

import concourse.tile as tile
from concourse import bacc, mybir
from concourse.bass import ds, ts
from concourse.bass_utils import run_bass_kernel_spmd

BF16 = ml_dtypes.bfloat16

B, M, D_IN, D_H, D_OUT = 512, 32, 1024, 4096, 1024
N_CORES = 8
SLOTS = M // N_CORES          # 4 slots per core
P = 128                       # SBUF partitions / PE array size
KI = D_IN // P                # 8 contraction chunks in layer 1
KH = D_H // P                 # 32 contraction chunks in layer 2
NB = B // P                   # 4 output row-blocks in layer 2
F = 512                       # moving free dim == one PSUM bank of fp32
ND = D_OUT // F               # 2 output column halves in layer 2

_NC_CACHE = {}


def build_nc(reps=1):
    """reps>1 repeats the whole computation in-NEFF (for timing-slope
    measurement in test.py; the graded kernel always uses reps=1)."""
    nc = bacc.Bacc("TRN2", target_bir_lowering=False, debug=False,
                   num_devices=N_CORES)
    bf = mybir.dt.bfloat16
    f32 = mybir.dt.float32
    xT = nc.dram_tensor("xT", [SLOTS, P, KI * B], bf, kind="ExternalInput").ap()
    w1 = nc.dram_tensor("w1", [SLOTS, KH, P, KI * P], bf, kind="ExternalInput").ap()
    b1 = nc.dram_tensor("b1", [SLOTS, P, KH], f32, kind="ExternalInput").ap()
    w2 = nc.dram_tensor("w2", [SLOTS, KH, P, D_OUT], bf, kind="ExternalInput").ap()
    b2 = nc.dram_tensor("b2", [SLOTS, P, D_OUT], f32, kind="ExternalInput").ap()
    out = nc.dram_tensor("out", [SLOTS, B, D_OUT], f32, kind="ExternalOutput").ap()

    with tile.TileContext(nc) as tc:
        with (
            tc.tile_pool(name="xt_pool", bufs=2) as xt_pool,
            tc.tile_pool(name="w1_pool", bufs=4) as w1_pool,
            tc.tile_pool(name="w2_pool", bufs=1) as w2_pool,
            tc.tile_pool(name="h_pool", bufs=2) as h_pool,
            tc.tile_pool(name="bias_pool", bufs=2) as bias_pool,
            tc.tile_pool(name="out_pool", bufs=4) as out_pool,
            tc.tile_pool(name="ps1_pool", bufs=3, space="PSUM") as ps1_pool,
            tc.tile_pool(name="ps2_pool", bufs=4, space="PSUM") as ps2_pool,
        ):
            for rep in range(reps):
              for s0 in range(SLOTS):
                s = s0 if (rep % 2 == 0) else (SLOTS - 1 - s0)
                rs = f"{rep}_{s}"
                xt = xt_pool.tile([P, KI * B], bf, name=f"xt_{rs}", tag="xt")
                for k in range(KI):
                    nc.sync.dma_start(out=xt[:, ts(k, B)], in_=xT[s, :, ts(k, B)])
                b1t = bias_pool.tile([P, KH], f32, name=f"b1_{rs}", tag="b1")
                nc.sync.dma_start(out=b1t[:], in_=b1[s])
                b2t = bias_pool.tile([P, D_OUT], f32, name=f"b2_{rs}", tag="b2")
                nc.sync.dma_start(out=b2t[:], in_=b2[s])
                w2t = w2_pool.tile([P, KH * D_OUT], bf, name=f"w2_{rs}", tag="w2")
                for j in range(KH):
                    nc.sync.dma_start(out=w2t[:, ts(j, D_OUT)], in_=w2[s, j])

                # Layer 1: h^T[dh, b] = relu(W1^T @ x^T + b1), bf16 out
                h = h_pool.tile([P, KH * B], bf, name=f"h_{rs}", tag="h")
                for j in range(KH):
                    w1t = w1_pool.tile([P, KI * P], bf, name=f"w1_{rs}_{j}", tag="w1")
                    nc.sync.dma_start(out=w1t[:], in_=w1[s, j])
                    ps1 = ps1_pool.tile([P, B], f32, name=f"ps1_{rs}_{j}", tag="ps1")
                    for k in range(KI):
                        nc.tensor.matmul(
                            ps1[:],
                            lhsT=w1t[:, ts(k, P)],
                            rhs=xt[:, ts(k, B)],
                            start=(k == 0),
                            stop=(k == KI - 1),
                        )
                    nc.scalar.activation(
                        h[:, ts(j, B)],
                        ps1[:],
                        mybir.ActivationFunctionType.Relu,
                        bias=b1t[:, ds(j, 1)],
                        scale=1.0,
                    )

                # Layer 2: out[b, o] = h^T-slice^T @ W2 + b2
                for bb in range(NB):
                    pss = [
                        ps2_pool.tile([P, F], f32, name=f"ps2_{rs}_{bb}_{d}", tag="ps2")
                        for d in range(ND)
                    ]
                    for j in range(KH):
                        for d in range(ND):
                            nc.tensor.matmul(
                                pss[d][:],
                                lhsT=h[:, ds(j * B + bb * P, P)],
                                rhs=w2t[:, ds(j * D_OUT + d * F, F)],
                                start=(j == 0),
                                stop=(j == KH - 1),
                            )
                    osb = out_pool.tile([P, D_OUT], f32, name=f"osb_{rs}_{bb}", tag="osb")
                    for d in range(ND):
                        nc.vector.tensor_tensor(
                            out=osb[:, ts(d, F)],
                            in0=pss[d][:],
                            in1=b2t[:, ts(d, F)],
                            op=mybir.AluOpType.add,
                        )
                    nc.sync.dma_start(out=out[s, ts(bb, P)], in_=osb[:])
    nc.compile()
    return nc


def prep_in_maps(memory_hidden_states, W1, b1, W2, b2):
    """Host-side packing of full inputs into per-core, SBUF-layout arrays."""
    x = np.asarray(memory_hidden_states, dtype=np.float32)
    W1 = np.asarray(W1, dtype=np.float32)
    b1 = np.asarray(b1, dtype=np.float32)
    W2 = np.asarray(W2, dtype=np.float32)
    b2 = np.asarray(b2, dtype=np.float32)

    # x: [B, M, D_IN] -> x^T per slot: [M, P, KI*B]; [m, p, k*B+b] = x[b, m, k*P+p]
    xT = np.ascontiguousarray(
        x.astype(BF16).transpose(1, 2, 0)       # [M, D_IN, B]
         .reshape(M, KI, P, B)
         .transpose(0, 2, 1, 3)                 # [M, P, KI, B]
         .reshape(M, P, KI * B)
    )
    # W1: [M, D_IN, D_H] -> [M, KH, P(=din%128), KI*128]
    w1 = np.ascontiguousarray(
        W1.astype(BF16).reshape(M, KI, P, KH, P)
          .transpose(0, 3, 2, 1, 4)             # [M, KH(j), P(p), KI(k), 128(c)]
          .reshape(M, KH, P, KI * P)
    )
    # b1: [M, D_H] -> [M, P, KH] (per-partition bias per dh-block)
    b1p = np.ascontiguousarray(b1.reshape(M, KH, P).transpose(0, 2, 1))
    # W2: [M, D_H, D_OUT] -> [M, KH, P, D_OUT] (pure reshape)
    w2 = np.ascontiguousarray(W2.astype(BF16).reshape(M, KH, P, D_OUT))
    # b2: [M, D_OUT] -> [M, P, D_OUT] broadcast across partitions
    b2p = np.ascontiguousarray(np.broadcast_to(b2[:, None, :], (M, P, D_OUT)))

    in_maps = []
    for c in range(N_CORES):
        sl = slice(c * SLOTS, (c + 1) * SLOTS)
        in_maps.append({
            "xT": xT[sl],
            "w1": w1[sl],
            "b1": b1p[sl],
            "w2": w2[sl],
            "b2": b2p[sl],
        })
    return in_maps


def assemble_out(results):
    """results: list of 8 dicts with 'out' [SLOTS, B, D_OUT] -> [B, M, D_OUT]."""
    full = np.concatenate([results[c]["out"] for c in range(N_CORES)], axis=0)
    return np.ascontiguousarray(full.transpose(1, 0, 2))


def kernel(memory_hidden_states, W1, b1, W2, b2):
    if "nc" not in _NC_CACHE:
        _NC_CACHE["nc"] = build_nc()
    nc = _NC_CACHE["nc"]
    in_maps = prep_in_maps(memory_hidden_states, W1, b1, W2, b2)
    res = run_bass_kernel_spmd(nc, in_maps, list(range(N_CORES)))
    return assemble_out(res.results)


# revision 10
# speedup vs baseline: 138.7792x; 2.7806x over previous
"""Trainium2 Bass kernel for nn_CompressNetwork (grouped per-slot MLP).

Reference computation (per slot m of M=32):
    h   = relu(x[:, m, :] @ W1[m] + b1[m])      # [B, D_H]
    out = h @ W2[m] + b2[m]                     # [B, D_OUT]

Sharding: slots are split across 8 NeuronCores (4 slots/core), expert-parallel.
No cross-core communication. Compute is done in bf16 with fp32 PSUM
accumulation (rel err ~1e-3, well inside tolerance, 4x the fp32 PE rate).

Device-side layout trick: layer 1 computes h^T (h transposed) so that
 - layer 1 stationary operand is W1 in its *natural* [D_IN, D_H] layout,
 - layer 1 moving operand is x^T (pre-transposed on host, free),
 - b1 becomes a per-partition bias => fused into the ScalarE relu,
 - layer 2 stationary operand is a [128,128] slice of h^T (already in SBUF),
 - layer 2 moving operand is W2 in its *natural* [D_H, D_OUT] layout,
so no on-device transposes are needed anywhere.
"""

import numpy as np
import ml_dtypes

import concourse.tile as tile
from concourse import bacc, mybir
from concourse.bass import ds, ts
from concourse.bass_utils import run_bass_kernel_spmd

BF16 = ml_dtypes.bfloat16

B, M, D_IN, D_H, D_OUT = 512, 32, 1024, 4096, 1024
N_CORES = 8
SLOTS = M // N_CORES          # 4 slots per core
P = 128                       # SBUF partitions / PE array size
KI = D_IN // P                # 8 contraction chunks in layer 1
KH = D_H // P                 # 32 contraction chunks in layer 2
NB = B // P                   # 4 output row-blocks in layer 2
F = 512                       # moving free dim == one PSUM bank of fp32
ND = D_OUT // F               # 2 output column halves in layer 2
G = 4                         # dh-blocks per W1/W2 DMA (1 MiB batches)

_NC_CACHE = {}


def build_nc(reps=1):
    """reps>1 repeats the whole computation in-NEFF (for timing-slope
    measurement in test.py; the graded kernel always uses reps=1)."""
    nc = bacc.Bacc("TRN2", target_bir_lowering=False, debug=False,
                   num_devices=N_CORES)
    bf = mybir.dt.bfloat16
    f32 = mybir.dt.float32
    xT = nc.dram_tensor("xT", [SLOTS, P, KI * B], bf, kind="ExternalInput").ap()
    w1 = nc.dram_tensor("w1", [SLOTS, KH // G, P, G * KI * P], bf,
                        kind="ExternalInput").ap()
    b1 = nc.dram_tensor("b1", [SLOTS, P, KH], f32, kind="ExternalInput").ap()
    w2 = nc.dram_tensor("w2", [SLOTS, KH // G, P, G * D_OUT], bf,
                        kind="ExternalInput").ap()
    b2 = nc.dram_tensor("b2", [SLOTS, P, D_OUT], f32, kind="ExternalInput").ap()
    out = nc.dram_tensor("out", [SLOTS, B, D_OUT], f32, kind="ExternalOutput").ap()

    with tile.TileContext(nc) as tc:
        with (
            tc.tile_pool(name="xt_pool", bufs=2) as xt_pool,
            tc.tile_pool(name="w1_pool", bufs=3) as w1_pool,
            tc.tile_pool(name="w2_pool", bufs=1) as w2_pool,
            tc.tile_pool(name="h_pool", bufs=2) as h_pool,
            tc.tile_pool(name="bias_pool", bufs=2) as bias_pool,
            tc.tile_pool(name="out_pool", bufs=3) as out_pool,
            tc.tile_pool(name="ps1_pool", bufs=3, space="PSUM") as ps1_pool,
            tc.tile_pool(name="ps2_pool", bufs=4, space="PSUM") as ps2_pool,
        ):
            for rep in range(reps):
              for s0 in range(SLOTS):
                s = s0 if (rep % 2 == 0) else (SLOTS - 1 - s0)
                rs = f"{rep}_{s}"
                xt = xt_pool.tile([P, KI * B], bf, name=f"xt_{rs}", tag="xt")
                nc.sync.dma_start(out=xt[:], in_=xT[s])
                bt = bias_pool.tile([P, D_OUT + KH], f32, name=f"bt_{rs}", tag="bt")
                b2t = bt[:, ds(0, D_OUT)]
                b1t = bt[:, ds(D_OUT, KH)]
                nc.sync.dma_start(out=b1t, in_=b1[s])
                nc.sync.dma_start(out=b2t, in_=b2[s])
                w2t = w2_pool.tile([P, KH * D_OUT], bf, name=f"w2_{rs}", tag="w2")
                for jg in range(KH // G):
                    nc.sync.dma_start(out=w2t[:, ts(jg, G * D_OUT)], in_=w2[s, jg])

                # Layer 1: h^T[dh, b] = relu(W1^T @ x^T + b1), bf16 out
                h = h_pool.tile([P, KH * B], bf, name=f"h_{rs}", tag="h")
                for jg in range(KH // G):
                    w1t = w1_pool.tile([P, G * KI * P], bf,
                                       name=f"w1_{rs}_{jg}", tag="w1")
                    nc.sync.dma_start(out=w1t[:], in_=w1[s, jg])
                    for g in range(G):
                        j = jg * G + g
                        ps1 = ps1_pool.tile([P, B], f32,
                                            name=f"ps1_{rs}_{j}", tag="ps1")
                        for k in range(KI):
                            nc.tensor.matmul(
                                ps1[:],
                                lhsT=w1t[:, ds((g * KI + k) * P, P)],
                                rhs=xt[:, ts(k, B)],
                                start=(k == 0),
                                stop=(k == KI - 1),
                            )
                        nc.scalar.activation(
                            h[:, ts(j, B)],
                            ps1[:],
                            mybir.ActivationFunctionType.Relu,
                            bias=b1t[:, ds(j, 1)],
                            scale=1.0,
                        )

                # Layer 2: out[b, o] = h^T-slice^T @ W2 + b2
                for bb in range(NB):
                    pss = [
                        ps2_pool.tile([P, F], f32, name=f"ps2_{rs}_{bb}_{d}", tag="ps2")
                        for d in range(ND)
                    ]
                    for j in range(KH):
                        for d in range(ND):
                            nc.tensor.matmul(
                                pss[d][:],
                                lhsT=h[:, ds(j * B + bb * P, P)],
                                rhs=w2t[:, ds(j * D_OUT + d * F, F)],
                                start=(j == 0),
                                stop=(j == KH - 1),
                            )
                    osb = out_pool.tile([P, D_OUT], f32, name=f"osb_{rs}_{bb}", tag="osb")
                    for d in range(ND):
                        nc.vector.tensor_tensor(
                            out=osb[:, ts(d, F)],
                            in0=pss[d][:],
                            in1=b2t[:, ts(d, F)],
                            op=mybir.AluOpType.add,
                        )
                    nc.sync.dma_start(out=out[s, ts(bb, P)], in_=osb[:])
    nc.compile()
    return nc


def prep_in_maps(memory_hidden_states, W1, b1, W2, b2):
    """Host-side packing of full inputs into per-core, SBUF-layout arrays."""
    x = np.asarray(memory_hidden_states, dtype=np.float32)
    W1 = np.asarray(W1, dtype=np.float32)
    b1 = np.asarray(b1, dtype=np.float32)
    W2 = np.asarray(W2, dtype=np.float32)
    b2 = np.asarray(b2, dtype=np.float32)

    # x: [B, M, D_IN] -> x^T per slot: [M, P, KI*B]; [m, p, k*B+b] = x[b, m, k*P+p]
    xT = np.ascontiguousarray(
        x.astype(BF16).transpose(1, 2, 0)       # [M, D_IN, B]
         .reshape(M, KI, P, B)
         .transpose(0, 2, 1, 3)                 # [M, P, KI, B]
         .reshape(M, P, KI * B)
    )
    # W1: [M, D_IN, D_H] -> [M, KH//G, P, G*KI*128]
    # [m, jg, p, (g*KI+k)*128+c] = W1[m, k*128+p, (jg*G+g)*128+c]
    w1 = np.ascontiguousarray(
        W1.astype(BF16).reshape(M, KI, P, KH // G, G, P)
          .transpose(0, 3, 2, 4, 1, 5)          # [M, jg, p, g, k, c]
          .reshape(M, KH // G, P, G * KI * P)
    )
    # b1: [M, D_H] -> [M, P, KH] (per-partition bias per dh-block)
    b1p = np.ascontiguousarray(b1.reshape(M, KH, P).transpose(0, 2, 1))
    # W2: [M, D_H, D_OUT] -> [M, KH//G, P, G*D_OUT]
    # [m, jg, p, g*D_OUT+o] = W2[m, (jg*G+g)*128+p, o]
    w2 = np.ascontiguousarray(
        W2.astype(BF16).reshape(M, KH // G, G, P, D_OUT)
          .transpose(0, 1, 3, 2, 4)             # [M, jg, p, g, o]
          .reshape(M, KH // G, P, G * D_OUT)
    )
    # b2: [M, D_OUT] -> [M, P, D_OUT] broadcast across partitions
    b2p = np.ascontiguousarray(np.broadcast_to(b2[:, None, :], (M, P, D_OUT)))

    in_maps = []
    for c in range(N_CORES):
        sl = slice(c * SLOTS, (c + 1) * SLOTS)
        in_maps.append({
            "xT": xT[sl],
            "w1": w1[sl],
            "b1": b1p[sl],
            "w2": w2[sl],
            "b2": b2p[sl],
        })
    return in_maps


def assemble_out(results):
    """results: list of 8 dicts with 'out' [SLOTS, B, D_OUT] -> [B, M, D_OUT]."""
    full = np.concatenate([results[c]["out"] for c in range(N_CORES)], axis=0)
    return np.ascontiguousarray(full.transpose(1, 0, 2))


def kernel(memory_hidden_states, W1, b1, W2, b2):
    if "nc" not in _NC_CACHE:
        _NC_CACHE["nc"] = build_nc()
    nc = _NC_CACHE["nc"]
    in_maps = prep_in_maps(memory_hidden_states, W1, b1, W2, b2)
    res = run_bass_kernel_spmd(nc, in_maps, list(range(N_CORES)))
    return assemble_out(res.results)
